# revision 49
# baseline (speedup 1.0000x reference)
"""Trainium2 Bass kernel for nn_MultiHeadAttention (B=2, L=2048, D=1024, H=16).

Sharding: 8 cores = 2 batches (data parallel) x 4 head-groups (tensor
parallel, 4 heads / 256 features per core).  Each core computes its partial
o-proj output; the host sums the 4 partials per batch and adds the output
bias (the "all-reduce" of the unshard step).

Device-side layout is feature-major end to end so no on-device transposes
are needed: the host ships x already transposed to [d, t] and the weights
pre-sliced/transposed.  Softmax denominators come for free from ones
columns appended to V (rows 64-67 of each AV psum accumulator, one-hot per
(pair, head)), so each PSUM bank hosts exactly one accumulation group.

Schedule: attention for the first group starts as soon as k-span0 and
q-spans 0/1 are projected; the remaining k/v spans are projected inside the
first group's chunk loop.  Scores for both heads of a chunk share one
[128, 1024] psum tile per q-span (h0 cols 0-511, h1 cols 512-1023) so the
two K=64 matmuls land on disjoint PE row groups and stream concurrently,
and each exp covers both heads in one ACTIVATE.  Normalization bounces the
reciprocal denominator rows through DRAM for the partition broadcast (in
512-wide halves so the tail can pipeline), multiplies on DVE, and only h1
needs a partition-shift DMA into the combined avc tile.  The output is
stored bf16 (host accumulates in fp32) to halve the tail's store traffic.
"""

import os
import sys

import ml_dtypes
import numpy as np

for _p in ("/opt/trn_rl_repo", "/root/.axon_site/_ro/trn_rl_repo"):
    if os.path.isdir(_p) and _p not in sys.path:
        sys.path.append(_p)

import concourse.bass as bass
import concourse.mybir as mybir
import concourse.tile as tile
from concourse import bacc
from concourse.bass_utils import run_bass_kernel_spmd

# Problem shape (hardcoded per contract)
B, L, D = 2, 2048, 1024
H, DH = 16, 64
N_CORES = 8
GROUPS = 4            # cores per batch (head-parallel)
HL = H // GROUPS      # 4 local heads per core
F = HL * DH           # 256 local features per core
KC = 128              # attention contraction chunk (k tokens)
NKC = L // KC         # 16 chunks
SPAN = 512            # matmul free-dim span
NSPAN = L // SPAN     # 4 spans
DC = 128              # projection contraction chunk
NDC = D // DC         # 8 chunks

F32 = mybir.dt.float32
F32R = mybir.dt.float32r
BF16 = mybir.dt.bfloat16
EXP = mybir.ActivationFunctionType.Exp
LOG = mybir.ActivationFunctionType.Ln


def build_bass(fix_waits=True):
    nc = bacc.Bacc("TRN2", target_bir_lowering=False)

    xq = nc.dram_tensor("xq", [128, NSPAN, NDC, SPAN], BF16, kind="ExternalInput")
    xk = nc.dram_tensor("xk", [128, NSPAN, NDC, SPAN], BF16, kind="ExternalInput")
    xv = nc.dram_tensor("xv", [128, NSPAN, NDC, SPAN], BF16, kind="ExternalInput")
    wq = nc.dram_tensor("wq", [128, NDC, F], BF16, kind="ExternalInput")
    wk = nc.dram_tensor("wk", [128, NDC, F], BF16, kind="ExternalInput")
    wv = nc.dram_tensor("wv", [128, NDC, F], BF16, kind="ExternalInput")
    wo = nc.dram_tensor("wo", [128, 2, D], BF16, kind="ExternalInput")
    bqt = nc.dram_tensor("bqt", [128, 2], F32, kind="ExternalInput")
    bkt = nc.dram_tensor("bkt", [128, 2], F32, kind="ExternalInput")
    bvt = nc.dram_tensor("bvt", [1, F], BF16, kind="ExternalInput")
    out = nc.dram_tensor("out", [D, L], BF16, kind="ExternalOutput")
    scratch = nc.dram_tensor("scratch_recip", [2, 4, 2 * SPAN], F32)

    with tile.TileContext(nc) as tc:
        _emit(nc, tc, xq, xk, xv, wq, wk, wv, wo, bqt, bkt, bvt, out, scratch)
    # Bacc lowering: splits multi-wait sync_infos into EventSemaphores (the
    # walrus ISA structs have a single sync-wait slot), inserts gpsimd
    # library loads and ACT table loads.
    nc.compile()
    return nc


def _emit(nc, tc, xq, xk, xv, wq, wk, wv, wo, bqt, bkt, bvt, out, scratch):
    with (
        tc.tile_pool(name="consts", bufs=1) as consts,
        tc.tile_pool(name="weights", bufs=1) as weights,
        tc.tile_pool(name="persist", bufs=1) as persist,
        tc.tile_pool(name="xin", bufs=2) as xin,
        tc.tile_pool(name="ptp", bufs=6) as ptp,
        tc.tile_pool(name="rbp", bufs=4) as rbp,
        tc.tile_pool(name="outp", bufs=3) as outp,
        tc.tile_pool(name="mm", bufs=2, space="PSUM") as mmp,
        tc.tile_pool(name="acc", bufs=1, space="PSUM") as accp,
    ):
        # ---- constants ----
        ones_row = consts.tile([1, 128], BF16, tag="ones_row", name="ones_row")
        nc.vector.memset(ones_row[:], 1.0)

        # ---- weights / biases to SBUF (k first: first matmuls need it) ----
        wk_sb = weights.tile([128, NDC, F], BF16, tag="wk", name="wk_sb")
        wq_sb = weights.tile([128, NDC, F], BF16, tag="wq", name="wq_sb")
        wv_sb = weights.tile([128, NDC, F], BF16, tag="wv", name="wv_sb")
        wo2_sb = weights.tile([128, 2, D], BF16, tag="wo", name="wo2_sb")
        bq_sb = consts.tile([128, 2], F32, tag="bq", name="bq_sb")
        bk_sb = consts.tile([128, 2], F32, tag="bk", name="bk_sb")
        bv_sb = consts.tile([1, F], BF16, tag="bv", name="bv_sb")
        # first two o-chunks of wk land first so the first k-proj matmuls
        # start ~8us earlier than a monolithic 0.5MB load would allow
        nc.sync.dma_start(out=wk_sb[:, 0:2, :], in_=wk[:, 0:2, :])

        # ---- persistent activation tiles ----
        # qhT/khT: [f(128 = 2 heads of pair), t] feature-major
        # vh: [t, 64+pad] token-major per (pair, head, span); col 64+idx = ones
        # (idx = 2p+h), so each AV psum carries its denominator row at a
        # distinct partition 64+idx.
        qhT = {(p, s): persist.tile([128, SPAN], BF16, tag=f"qhT{p}{s}", name=f"qhT{p}{s}")
               for p in range(2) for s in range(NSPAN)}
        khT = {(p, s): persist.tile([128, SPAN], BF16, tag=f"khT{p}{s}", name=f"khT{p}{s}")
               for p in range(2) for s in range(NSPAN)}
        vh = {(p, h, s): persist.tile([128, 4, 68], BF16, tag=f"vh{p}{h}{s}", name=f"vh{p}{h}{s}")
              for p in range(2) for h in range(2) for s in range(NSPAN)}
        # unnormalized attention values, bf16, per (pair, head, spanpair)
        avs = {(p, h, sp): persist.tile([64, 2 * SPAN], BF16, tag=f"avs{p}{h}{sp}", name=f"avs{p}{h}{sp}")
               for p in range(2) for h in range(2) for sp in range(2)}
        # denominators at rows 64-67 (one-hot per head idx); per (spanpair,
        # pair, head) so the two heads' drains stage in parallel and the
        # pack DMAs gather the real rows without a merge add
        den = {(sp, p, h): persist.tile([68, 2 * SPAN], F32, tag=f"den{sp}{p}{h}", name=f"den{sp}{p}{h}")
               for sp in range(2) for p in range(2) for h in range(2)}
        # normalized pair tiles (h0 rows 0-63, h1 rows 64-127) for K=128 o-proj
        avc = {(p, sp): persist.tile([128, 2 * SPAN], BF16, tag=f"avc{p}{sp}", name=f"avc{p}{sp}")
               for p in range(2) for sp in range(2)}

        for (p, h, s), t in vh.items():
            idx = 2 * p + h
            nc.vector.memset(t[:, :, 64:68], 0.0)
            nc.vector.memset(t[:, :, 64 + idx:65 + idx], 1.0)

        # ---- projection pieces (emitted interleaved with attention) ----
        def dma_x(dram, nm, s):
            t = xin.tile([128, NDC, SPAN], BF16, tag=nm, name=nm, bufs=2)
            nc.sync.dma_start(out=t[:], in_=dram[:, s])
            return t

        def fproj_pair(w_sb, b_sb, x_t, dst, s, p):
            # feature-major projection (q/k): out [f 128, t 512] for one pair
            ps = mmp.tile([128, 2 * SPAN], F32, tag="mm", name="mm")
            for o in range(NDC):
                nc.tensor.matmul(
                    ps[:, 0:SPAN],
                    w_sb[:, o, p * 128:(p + 1) * 128],
                    x_t[:, o, :],
                    start=(o == 0), stop=(o == NDC - 1),
                )
            nc.vector.tensor_scalar_add(
                dst[(p, s)][:], ps[:, 0:SPAN], b_sb[:, p:p + 1]
            )

        def kproj(s, x_t=None):
            x_t = x_t if x_t is not None else dma_x(xk, "xk", s)
            for p in range(2):
                fproj_pair(wk_sb, bk_sb, x_t, khT, s, p)

        def qproj(s, x_t=None):
            x_t = x_t if x_t is not None else dma_x(xq, "xq", s)
            for p in range(2):
                fproj_pair(wq_sb, bq_sb, x_t, qhT, s, p)

        def vproj_tt(xv_t, s, tt):
            # v proj token tile: out [t 128, f 256], bias via K=1 matmul
            ps = mmp.tile([128, 2 * SPAN], F32, tag="mm", name="mm")
            for o in range(NDC):
                nc.tensor.matmul(
                    ps[:, 0:F],
                    xv_t[:, o, tt * 128:(tt + 1) * 128],
                    wv_sb[:, o, :],
                    start=(o == 0), stop=False,
                )
            nc.tensor.matmul(
                ps[:, 0:F], ones_row[:], bv_sb[:],
                start=False, stop=True,
            )
            for p in range(2):
                for h in range(2):
                    nc.vector.tensor_copy(
                        vh[(p, h, s)][:, tt, 0:64],
                        ps[:, p * 128 + h * 64:p * 128 + (h + 1) * 64],
                    )

        def vproj(s):
            xv_t = dma_x(xv, "xv", s)
            for tt in range(SPAN // 128):
                vproj_tt(xv_t, s, tt)

        # ---- attention (flash over k chunks) ----
        av_live = {}

        def attn_chunks(sp, p, c_lo, c_hi):
            if (sp, p) not in av_live:
                av_live[(sp, p)] = {
                    h: accp.tile([128, 2 * SPAN], F32, tag=f"av{h}", name=f"av{h}")
                    for h in range(2)}
            av = av_live[(sp, p)]
            for c in range(c_lo, c_hi):   # k chunks of 128
                ks, cc = c // 4, c % 4
                ko = cc * 128
                # scores: one psum tile per q-span j holding both heads
                # (h0 cols 0-511, h1 cols 512-1023); the two K=64 matmuls
                # occupy disjoint PE row groups and stream concurrently.
                st = {j: mmp.tile([128, 2 * SPAN], F32, tag="mm", name="mm")
                      for j in range(2)}
                for j in range(2):
                    for h in range(2):
                        r0 = h * 64
                        nc.tensor.matmul(
                            st[j][:, h * SPAN:(h + 1) * SPAN],
                            khT[(p, ks)][r0:r0 + 64, ko:ko + 128],
                            qhT[(p, sp * 2 + j)][r0:r0 + 64, :],
                            start=True, stop=True,
                            tile_position=(r0, 0),
                        )
                pt = {}
                for j in range(2):
                    ptt = ptp.tile([128, 2 * SPAN], BF16, tag="pt", name="pt")
                    nc.scalar.activation(ptt[:], st[j][:], EXP, scale=0.125)
                    pt[j] = ptt
                for j in range(2):  # AV + one-hot denominator rows 64-67
                    for h in range(2):
                        nc.tensor.matmul(
                            av[h][0:68, j * SPAN:(j + 1) * SPAN],
                            vh[(p, h, ks)][:, cc, :],
                            pt[j][:, h * SPAN:(h + 1) * SPAN],
                            start=(c == 0), stop=(c == NKC - 1),
                        )
            if c_hi == NKC:
                # drain: values (bf16 cast); accumulate one-hot denominator
                # blocks (rows 64-67) into the shared tile.  The last group
                # drains on the (by then idle) scalar engine so the tail's
                # DVE chain only carries the reciprocal + multiplies.
                last = (sp == 1 and p == 1)
                for h in range(2):
                    # h copies split across DVE/ACT at the last group so the
                    # two heads' denominator rows stage in parallel
                    cp = (nc.scalar.copy if (last and h == 1)
                          else nc.vector.tensor_copy)
                    cp(avs[(p, h, sp)][:], av[h][0:64, :])
                    cp(den[(sp, p, h)][64:68, :], av[h][64:68, :])
                del av_live[(sp, p)]

        def normalize(sp, p):
            # reciprocal of this pair's denominator rows, DRAM-bounced for
            # the partition broadcast; multiply on DVE.  h0 lands directly
            # in avc rows 0-63 (lane-aligned); h1 is multiplied in place
            # and partition-shifted into rows 64-127 by a small sbuf DMA.
            # Split in 512-wide halves so the tail can pipeline o-proj.
            # repack the pair's two 1024-wide denominator rows (one per
            # head tile — DMAs have no partition-alignment limits) into
            # [4, 512] so ONE free-size-bound DVE reciprocal (3.3us)
            # covers what two 512-wide ones would (6.6us serial)
            denp = rbp.tile([4, SPAN], F32, tag="denp", name="denp")
            for half in range(2):
                hs = slice(half * SPAN, (half + 1) * SPAN)
                nc.sync.dma_start(
                    out=denp[2 * half:2 * half + 1, :],
                    in_=den[(sp, p, 0)][64 + 2 * p:65 + 2 * p, hs])
                nc.sync.dma_start(
                    out=denp[2 * half + 1:2 * half + 2, :],
                    in_=den[(sp, p, 1)][65 + 2 * p:66 + 2 * p, hs])
            nc.vector.reciprocal(denp[:], denp[:])
            for half in range(2):
                sl = slice(half * SPAN, (half + 1) * SPAN)
                nc.sync.dma_start(out=scratch[sp, 2 * p:2 * p + 2, sl],
                                  in_=denp[2 * half:2 * half + 2, :])
                for h in range(2):
                    rb = rbp.tile([64, SPAN], F32, tag=f"rb{h}", name=f"rb{h}")
                    nc.sync.dma_start(
                        out=rb[:],
                        in_=scratch[sp, 2 * p + h, sl].partition_broadcast(64)
                    )
                    if h == 0:
                        nc.vector.tensor_mul(
                            avc[(p, sp)][0:64, sl], avs[(p, 0, sp)][:, sl], rb[:])
                    else:
                        nc.vector.tensor_mul(
                            avs[(p, 1, sp)][:, sl], avs[(p, 1, sp)][:, sl], rb[:])
                        nc.sync.dma_start(
                            out=avc[(p, sp)][64:128, sl],
                            in_=avs[(p, 1, sp)][:, sl],
                        )

        def oproj_sp(sp):
            # o-proj with the Wo slice stationary, producing the TRANSPOSED
            # output out_T[d, q] = sum_f wo[f, d] * avc[f, q] (host untransposes).
            # Each d-chunk reuses one weight load across both q-spans per
            # pair; drain copies alternate DVE/ACT so neither engine is the
            # tail bottleneck.
            for dch in range(NDC):
                pool, tag = [(accp, "av0"), (accp, "av1"),
                             (mmp, "mm"), (mmp, "mm")][dch % 4]
                o_ps = pool.tile([128, 2 * SPAN], F32, tag=tag, name="ops")
                for p in range(2):
                    for j in range(2):
                        nc.tensor.matmul(
                            o_ps[:, j * SPAN:(j + 1) * SPAN],
                            wo2_sb[:, p, dch * 128:(dch + 1) * 128],
                            avc[(p, sp)][:, j * SPAN:(j + 1) * SPAN],
                            start=(p == 0), stop=(p == 1),
                        )
                o_sbT = outp.tile([128, 2 * SPAN], BF16, tag="osb", name="osb")
                if dch % 2 == 0:
                    nc.vector.tensor_copy(o_sbT[:], o_ps[:])
                else:
                    nc.scalar.copy(o_sbT[:], o_ps[:])
                nc.sync.dma_start(
                    out=out[dch * 128:(dch + 1) * 128,
                            sp * 2 * SPAN:(sp + 1) * 2 * SPAN],
                    in_=o_sbT[:])

        # ---- schedule ----
        # minimal prefix so the first exp lands early: k0, q0, q1 feed the
        # first group's scores; remaining k/v/q spans ride inside the first
        # two groups' chunk loops.
        xk0_t = xin.tile([128, NDC, SPAN], BF16, tag="xk", name="xk", bufs=2)
        nc.sync.dma_start(out=xk0_t[:, 0:2, :], in_=xk[:, 0, 0:2, :])
        nc.sync.dma_start(out=wk_sb[:, 2:NDC, :], in_=wk[:, 2:NDC, :])
        nc.sync.dma_start(out=xk0_t[:, 2:NDC, :], in_=xk[:, 0, 2:NDC, :])
        nc.sync.dma_start(out=wq_sb[:], in_=wq[:])
        nc.sync.dma_start(out=bk_sb[:], in_=bkt[:])
        nc.sync.dma_start(out=bq_sb[:], in_=bqt[:])
        kproj(0, xk0_t)
        qproj(0)
        qproj(1)
        nc.sync.dma_start(out=wv_sb[:], in_=wv[:])
        nc.sync.dma_start(out=bv_sb[:], in_=bvt[:])
        vproj(0)
        # group (0,0): one small projection unit per chunk so the single-exp
        # lookahead amortizes the PSUM mm-slot each unit borrows.  Weight
        # producers stay >= 1 chunk ahead of their consuming matmuls.
        xk_t = {}
        xv_t = {}
        units = {
            1: [("k", 1, 0)], 2: [("k", 1, 1)],
            3: [("v", 1, 0)], 4: [("v", 1, 1)], 5: [("v", 1, 2)],
            6: [("v", 1, 3), ("k", 2, 0)],
            7: [("k", 2, 1), ("v", 2, 0)],
            8: [("v", 2, 1)], 9: [("v", 2, 2)],
            10: [("v", 2, 3), ("k", 3, 0)],
            11: [("k", 3, 1), ("v", 3, 0)],
            12: [("v", 3, 1)], 13: [("v", 3, 2)],
            14: [("v", 3, 3), ("q", 2, 0)],
            15: [("q", 2, 1), ("q", 3, 0)],
        }
        for c in range(NKC):
            for kind, s, i in units.get(c, []):
                if kind == "k":
                    if i == 0:
                        xk_t[s] = dma_x(xk, "xk", s)
                    fproj_pair(wk_sb, bk_sb, xk_t[s], khT, s, i)
                elif kind == "q":
                    if i == 0:
                        xk_t[("q", s)] = dma_x(xq, "xq", s)
                    fproj_pair(wq_sb, bq_sb, xk_t[("q", s)], qhT, s, i)
                else:
                    if i == 0:
                        xv_t[s] = dma_x(xv, "xv", s)
                    vproj_tt(xv_t[s], s, i)
            attn_chunks(0, 0, c, c + 1)
        normalize(0, 0)
        # finish q3 (second pair) before the transition work
        fproj_pair(wq_sb, bq_sb, xk_t[("q", 3)], qhT, 3, 1)
        attn_chunks(0, 1, 0, 16)
        normalize(0, 1)
        nc.sync.dma_start(out=wo2_sb[:], in_=wo[:])
        attn_chunks(1, 0, 0, 16)
        normalize(1, 0)
        attn_chunks(1, 1, 0, 16)
        # tail: the sp0 o-proj units flow immediately (avc ready) while the
        # normalize(1, 1) chain runs; sp1 units follow as avc(., 1) lands
        normalize(1, 1)
        oproj_sp(0)
        oproj_sp(1)


def shard_inputs(q, k, v, Wq, bq, Wk, bk, Wv, bv, Wo, bo):
    """Host-side shard + layout prep. Returns list of 8 per-core input dicts."""
    def chunk_pf(a2d, pdim, dtype=np.float32):
        # (n*pdim, f) -> (pdim, n, f) with row r = o*pdim + p
        n, f = a2d.shape
        return np.ascontiguousarray(
            a2d.reshape(n // pdim, pdim, f).transpose(1, 0, 2)
        ).astype(dtype)

    def chunk_span(a2d, dtype):
        # (1024, 2048) -> [128, NSPAN, NDC, SPAN] so each span slice is a
        # contiguous 8 KiB per partition (fast DMA)
        x = chunk_pf(a2d, 128, dtype)          # [128, NDC, L]
        return np.ascontiguousarray(
            x.reshape(128, NDC, NSPAN, SPAN).transpose(0, 2, 1, 3))

    in_maps = []
    for core in range(N_CORES):
        b = core // GROUPS
        g = core % GROUPS
        fs = slice(g * F, (g + 1) * F)
        bf = ml_dtypes.bfloat16
        m = {
            "xq": chunk_span(np.ascontiguousarray(q[b].T), bf),
            "xk": chunk_span(np.ascontiguousarray(k[b].T), bf),
            "xv": chunk_span(np.ascontiguousarray(v[b].T), bf),
            "wq": chunk_pf(np.ascontiguousarray(Wq[fs, :].T), 128, bf),
            "wk": chunk_pf(np.ascontiguousarray(Wk[fs, :].T), 128, bf),
            "wv": chunk_pf(np.ascontiguousarray(Wv[fs, :].T), 128, bf),
            "wo": chunk_pf(np.ascontiguousarray(Wo[:, fs].T), 128, bf),
            "bqt": np.ascontiguousarray(bq[fs].reshape(2, 128).T).astype(np.float32),
            "bkt": np.ascontiguousarray(bk[fs].reshape(2, 128).T).astype(np.float32),
            "bvt": np.ascontiguousarray(bv[fs].reshape(1, F)).astype(ml_dtypes.bfloat16),
        }
        in_maps.append(m)
    return in_maps


_NC_CACHE = None


def _get_nc():
    global _NC_CACHE
    if _NC_CACHE is None:
        _NC_CACHE = build_bass()
    return _NC_CACHE


def run_spmd(inputs, trace=False, **kw):
    """Run the 8-core kernel; returns (full_output, BassKernelResults)."""
    q = np.asarray(inputs["q"], np.float32)
    k = np.asarray(inputs["k"], np.float32)
    v = np.asarray(inputs["v"], np.float32)
    in_maps = shard_inputs(
        q, k, v,
        np.asarray(inputs["Wq"], np.float32), np.asarray(inputs["bq"], np.float32),
        np.asarray(inputs["Wk"], np.float32), np.asarray(inputs["bk"], np.float32),
        np.asarray(inputs["Wv"], np.float32), np.asarray(inputs["bv"], np.float32),
        np.asarray(inputs["Wo"], np.float32), np.asarray(inputs["bo"], np.float32),
    )
    nc = _get_nc()
    res = run_bass_kernel_spmd(nc, in_maps, core_ids=list(range(N_CORES)),
                               trace=trace, **kw)
    bo = np.asarray(inputs["bo"], np.float32)
    full = np.empty((B, L, D), np.float32)
    for b in range(B):
        acc = res.results[b * GROUPS]["out"].astype(np.float32)
        for g in range(1, GROUPS):
            acc = acc + res.results[b * GROUPS + g]["out"].astype(np.float32)
        full[b] = acc.T + bo[None, :]   # device output is [D, L]
    return full, res


def kernel(**inputs):
    kpm = np.asarray(inputs["key_padding_mask"])
    if not bool(kpm.all()):
        return _numpy_fallback(**inputs)
    out, _ = run_spmd(inputs)
    return out


def _numpy_fallback(q, k, v, key_padding_mask, Wq, bq, Wk, bk, Wv, bv, Wo, bo):
    q = np.asarray(q, np.float32)
    k = np.asarray(k, np.float32)
    v = np.asarray(v, np.float32)
    B_, Lq, D_ = q.shape
    qh = (q @ np.asarray(Wq).T + bq).reshape(B_, Lq, H, DH).transpose(0, 2, 1, 3)
    kh = (k @ np.asarray(Wk).T + bk).reshape(B_, -1, H, DH).transpose(0, 2, 1, 3)
    vh = (v @ np.asarray(Wv).T + bv).reshape(B_, -1, H, DH).transpose(0, 2, 1, 3)
    s = np.einsum("bhqd,bhkd->bhqk", qh, kh) / np.sqrt(np.float32(DH))
    km = np.asarray(key_padding_mask)[:, None, None, :]
    s = np.where(km, s, -np.inf)
    s = s - s.max(-1, keepdims=True)
    p = np.exp(s)
    p = p / p.sum(-1, keepdims=True)
    o = np.einsum("bhqk,bhkd->bhqd", p, vh)
    o = o.transpose(0, 2, 1, 3).reshape(B_, Lq, D_)
    return (o @ np.asarray(Wo).T + bo).astype(np.float32)


# revision 50
# speedup vs baseline: 1.0184x; 1.0184x over previous
"""Trainium2 Bass kernel for nn_MultiHeadAttention (B=2, L=2048, D=1024, H=16).

Sharding: 8 cores = 2 batches (data parallel) x 4 head-groups (tensor
parallel, 4 heads / 256 features per core).  Each core computes its partial
o-proj output; the host sums the 4 partials per batch and adds the output
bias (the "all-reduce" of the unshard step).

Device-side layout is feature-major end to end so no on-device transposes
are needed: the host ships x already transposed to [d, t] and the weights
pre-sliced/transposed.  Softmax denominators come for free from ones
columns appended to V (rows 64-67 of each AV psum accumulator, one-hot per
(pair, head)), so each PSUM bank hosts exactly one accumulation group.

Schedule: attention for the first group starts as soon as k-span0 and
q-spans 0/1 are projected; the remaining k/v spans are projected inside the
first group's chunk loop.  Scores for both heads of a chunk share one
[128, 1024] psum tile per q-span (h0 cols 0-511, h1 cols 512-1023) so the
two K=64 matmuls land on disjoint PE row groups and stream concurrently,
and each exp covers both heads in one ACTIVATE.  Normalization bounces the
reciprocal denominator rows through DRAM for the partition broadcast (in
512-wide halves so the tail can pipeline), multiplies on DVE, and only h1
needs a partition-shift DMA into the combined avc tile.  The output is
stored bf16 (host accumulates in fp32) to halve the tail's store traffic.
"""

import os
import sys

import ml_dtypes
import numpy as np

for _p in ("/opt/trn_rl_repo", "/root/.axon_site/_ro/trn_rl_repo"):
    if os.path.isdir(_p) and _p not in sys.path:
        sys.path.append(_p)

import concourse.bass as bass
import concourse.mybir as mybir
import concourse.tile as tile
from concourse import bacc
from concourse.bass_utils import run_bass_kernel_spmd

# Problem shape (hardcoded per contract)
B, L, D = 2, 2048, 1024
H, DH = 16, 64
N_CORES = 8
GROUPS = 4            # cores per batch (head-parallel)
HL = H // GROUPS      # 4 local heads per core
F = HL * DH           # 256 local features per core
KC = 128              # attention contraction chunk (k tokens)
NKC = L // KC         # 16 chunks
SPAN = 512            # matmul free-dim span
NSPAN = L // SPAN     # 4 spans
DC = 128              # projection contraction chunk
NDC = D // DC         # 8 chunks

F32 = mybir.dt.float32
F32R = mybir.dt.float32r
BF16 = mybir.dt.bfloat16
EXP = mybir.ActivationFunctionType.Exp
LOG = mybir.ActivationFunctionType.Ln


def build_bass(fix_waits=True):
    nc = bacc.Bacc("TRN2", target_bir_lowering=False)

    xq = nc.dram_tensor("xq", [128, NSPAN, NDC, SPAN], BF16, kind="ExternalInput")
    xk = nc.dram_tensor("xk", [128, NSPAN, NDC, SPAN], BF16, kind="ExternalInput")
    xv = nc.dram_tensor("xv", [128, NSPAN, NDC, SPAN], BF16, kind="ExternalInput")
    wq = nc.dram_tensor("wq", [128, NDC, F], BF16, kind="ExternalInput")
    wk = nc.dram_tensor("wk", [128, NDC, F], BF16, kind="ExternalInput")
    wv = nc.dram_tensor("wv", [128, NDC, F], BF16, kind="ExternalInput")
    wo = nc.dram_tensor("wo", [128, 2, D], BF16, kind="ExternalInput")
    bqt = nc.dram_tensor("bqt", [128, 2], F32, kind="ExternalInput")
    bkt = nc.dram_tensor("bkt", [128, 2], F32, kind="ExternalInput")
    bvt = nc.dram_tensor("bvt", [1, F], BF16, kind="ExternalInput")
    out = nc.dram_tensor("out", [D, L], BF16, kind="ExternalOutput")
    scratch = nc.dram_tensor("scratch_recip", [2, 4, 2 * SPAN], F32)

    with tile.TileContext(nc) as tc:
        _emit(nc, tc, xq, xk, xv, wq, wk, wv, wo, bqt, bkt, bvt, out, scratch)
    # Bacc lowering: splits multi-wait sync_infos into EventSemaphores (the
    # walrus ISA structs have a single sync-wait slot), inserts gpsimd
    # library loads and ACT table loads.
    nc.compile()
    return nc


def _emit(nc, tc, xq, xk, xv, wq, wk, wv, wo, bqt, bkt, bvt, out, scratch):
    with (
        tc.tile_pool(name="consts", bufs=1) as consts,
        tc.tile_pool(name="weights", bufs=1) as weights,
        tc.tile_pool(name="persist", bufs=1) as persist,
        tc.tile_pool(name="xin", bufs=2) as xin,
        tc.tile_pool(name="ptp", bufs=6) as ptp,
        tc.tile_pool(name="rbp", bufs=4) as rbp,
        tc.tile_pool(name="outp", bufs=3) as outp,
        tc.tile_pool(name="mm", bufs=2, space="PSUM") as mmp,
        tc.tile_pool(name="acc", bufs=1, space="PSUM") as accp,
    ):
        # ---- constants ----
        ones_row = consts.tile([1, 128], BF16, tag="ones_row", name="ones_row")
        nc.vector.memset(ones_row[:], 1.0)

        # ---- weights / biases to SBUF (k first: first matmuls need it) ----
        wk_sb = weights.tile([128, NDC, F], BF16, tag="wk", name="wk_sb")
        wq_sb = weights.tile([128, NDC, F], BF16, tag="wq", name="wq_sb")
        wv_sb = weights.tile([128, NDC, F], BF16, tag="wv", name="wv_sb")
        wo2_sb = weights.tile([128, 2, D], BF16, tag="wo", name="wo2_sb")
        bq_sb = consts.tile([128, 2], F32, tag="bq", name="bq_sb")
        bk_sb = consts.tile([128, 2], F32, tag="bk", name="bk_sb")
        bv_sb = consts.tile([1, F], BF16, tag="bv", name="bv_sb")
        # first two o-chunks of wk land first so the first k-proj matmuls
        # start ~8us earlier than a monolithic 0.5MB load would allow
        nc.sync.dma_start(out=wk_sb[:, 0:2, :], in_=wk[:, 0:2, :])

        # ---- persistent activation tiles ----
        # qhT/khT: [f(128 = 2 heads of pair), t] feature-major
        # vh: [t, 64+pad] token-major per (pair, head, span); col 64+idx = ones
        # (idx = 2p+h), so each AV psum carries its denominator row at a
        # distinct partition 64+idx.
        qhT = {(p, s): persist.tile([128, SPAN], BF16, tag=f"qhT{p}{s}", name=f"qhT{p}{s}")
               for p in range(2) for s in range(NSPAN)}
        khT = {(p, s): persist.tile([128, SPAN], BF16, tag=f"khT{p}{s}", name=f"khT{p}{s}")
               for p in range(2) for s in range(NSPAN)}
        vh = {(p, h, s): persist.tile([128, 4, 68], BF16, tag=f"vh{p}{h}{s}", name=f"vh{p}{h}{s}")
              for p in range(2) for h in range(2) for s in range(NSPAN)}
        # unnormalized attention values, bf16, per (pair, head, spanpair)
        avs = {(p, h, sp): persist.tile([64, 2 * SPAN], BF16, tag=f"avs{p}{h}{sp}", name=f"avs{p}{h}{sp}")
               for p in range(2) for h in range(2) for sp in range(2)}
        # denominators at rows 64-67 (one-hot per head idx); per (spanpair,
        # pair, head) so the two heads' drains stage in parallel and the
        # pack DMAs gather the real rows without a merge add
        den = {(sp, p, h): persist.tile([68, 2 * SPAN], F32, tag=f"den{sp}{p}{h}", name=f"den{sp}{p}{h}")
               for sp in range(2) for p in range(2) for h in range(2)}
        # normalized pair tiles (h0 rows 0-63, h1 rows 64-127) for K=128 o-proj
        avc = {(p, sp): persist.tile([128, 2 * SPAN], BF16, tag=f"avc{p}{sp}", name=f"avc{p}{sp}")
               for p in range(2) for sp in range(2)}

        for (p, h, s), t in vh.items():
            idx = 2 * p + h
            nc.vector.memset(t[:, :, 64:68], 0.0)
            nc.vector.memset(t[:, :, 64 + idx:65 + idx], 1.0)

        # ---- projection pieces (emitted interleaved with attention) ----
        def dma_x(dram, nm, s):
            t = xin.tile([128, NDC, SPAN], BF16, tag=nm, name=nm, bufs=2)
            nc.sync.dma_start(out=t[:], in_=dram[:, s])
            return t

        def fproj_pair(w_sb, b_sb, x_t, dst, s, p):
            # feature-major projection (q/k): out [f 128, t 512] for one pair
            ps = mmp.tile([128, 2 * SPAN], F32, tag="mm", name="mm")
            for o in range(NDC):
                nc.tensor.matmul(
                    ps[:, 0:SPAN],
                    w_sb[:, o, p * 128:(p + 1) * 128],
                    x_t[:, o, :],
                    start=(o == 0), stop=(o == NDC - 1),
                )
            nc.vector.tensor_scalar_add(
                dst[(p, s)][:], ps[:, 0:SPAN], b_sb[:, p:p + 1]
            )

        def kproj(s, x_t=None):
            x_t = x_t if x_t is not None else dma_x(xk, "xk", s)
            for p in range(2):
                fproj_pair(wk_sb, bk_sb, x_t, khT, s, p)

        def qproj(s, x_t=None):
            x_t = x_t if x_t is not None else dma_x(xq, "xq", s)
            for p in range(2):
                fproj_pair(wq_sb, bq_sb, x_t, qhT, s, p)

        def vproj_tt(xv_t, s, tt):
            # v proj token tile: out [t 128, f 256], bias via K=1 matmul
            ps = mmp.tile([128, 2 * SPAN], F32, tag="mm", name="mm")
            for o in range(NDC):
                nc.tensor.matmul(
                    ps[:, 0:F],
                    xv_t[:, o, tt * 128:(tt + 1) * 128],
                    wv_sb[:, o, :],
                    start=(o == 0), stop=False,
                )
            nc.tensor.matmul(
                ps[:, 0:F], ones_row[:], bv_sb[:],
                start=False, stop=True,
            )
            for p in range(2):
                for h in range(2):
                    nc.vector.tensor_copy(
                        vh[(p, h, s)][:, tt, 0:64],
                        ps[:, p * 128 + h * 64:p * 128 + (h + 1) * 64],
                    )

        def vproj(s):
            xv_t = dma_x(xv, "xv", s)
            for tt in range(SPAN // 128):
                vproj_tt(xv_t, s, tt)

        # ---- attention (flash over k chunks) ----
        av_live = {}

        def attn_chunks(sp, p, c_lo, c_hi):
            if (sp, p) not in av_live:
                av_live[(sp, p)] = {
                    h: accp.tile([128, 2 * SPAN], F32, tag=f"av{h}", name=f"av{h}")
                    for h in range(2)}
            av = av_live[(sp, p)]
            for c in range(c_lo, c_hi):   # k chunks of 128
                ks, cc = c // 4, c % 4
                ko = cc * 128
                # scores: one psum tile per q-span j holding both heads
                # (h0 cols 0-511, h1 cols 512-1023); the two K=64 matmuls
                # occupy disjoint PE row groups and stream concurrently.
                st = {j: mmp.tile([128, 2 * SPAN], F32, tag="mm", name="mm")
                      for j in range(2)}
                for j in range(2):
                    for h in range(2):
                        r0 = h * 64
                        nc.tensor.matmul(
                            st[j][:, h * SPAN:(h + 1) * SPAN],
                            khT[(p, ks)][r0:r0 + 64, ko:ko + 128],
                            qhT[(p, sp * 2 + j)][r0:r0 + 64, :],
                            start=True, stop=True,
                            tile_position=(r0, 0),
                        )
                pt = {}
                for j in range(2):
                    ptt = ptp.tile([128, 2 * SPAN], BF16, tag="pt", name="pt")
                    nc.scalar.activation(ptt[:], st[j][:], EXP, scale=0.125)
                    pt[j] = ptt
                for j in range(2):  # AV + one-hot denominator rows 64-67
                    for h in range(2):
                        nc.tensor.matmul(
                            av[h][0:68, j * SPAN:(j + 1) * SPAN],
                            vh[(p, h, ks)][:, cc, :],
                            pt[j][:, h * SPAN:(h + 1) * SPAN],
                            start=(c == 0), stop=(c == NKC - 1),
                        )
            if c_hi == NKC:
                # drain: values (bf16 cast); accumulate one-hot denominator
                # blocks (rows 64-67) into the shared tile.  The last group
                # drains on the (by then idle) scalar engine so the tail's
                # DVE chain only carries the reciprocal + multiplies.
                for h in range(2):
                    # h0 drains on DVE, h1 on ACT (idle during the boundary)
                    # so the av banks release in half the time at every
                    # group transition
                    cp = nc.scalar.copy if h == 1 else nc.vector.tensor_copy
                    cp(avs[(p, h, sp)][:], av[h][0:64, :])
                    cp(den[(sp, p, h)][64:68, :], av[h][64:68, :])
                del av_live[(sp, p)]

        def normalize(sp, p):
            # reciprocal of this pair's denominator rows, DRAM-bounced for
            # the partition broadcast; multiply on DVE.  h0 lands directly
            # in avc rows 0-63 (lane-aligned); h1 is multiplied in place
            # and partition-shifted into rows 64-127 by a small sbuf DMA.
            # Split in 512-wide halves so the tail can pipeline o-proj.
            # repack the pair's two 1024-wide denominator rows (one per
            # head tile — DMAs have no partition-alignment limits) into
            # [4, 512] so ONE free-size-bound DVE reciprocal (3.3us)
            # covers what two 512-wide ones would (6.6us serial)
            denp = rbp.tile([4, SPAN], F32, tag="denp", name="denp")
            for half in range(2):
                hs = slice(half * SPAN, (half + 1) * SPAN)
                nc.sync.dma_start(
                    out=denp[2 * half:2 * half + 1, :],
                    in_=den[(sp, p, 0)][64 + 2 * p:65 + 2 * p, hs])
                nc.sync.dma_start(
                    out=denp[2 * half + 1:2 * half + 2, :],
                    in_=den[(sp, p, 1)][65 + 2 * p:66 + 2 * p, hs])
            nc.vector.reciprocal(denp[:], denp[:])
            for half in range(2):
                sl = slice(half * SPAN, (half + 1) * SPAN)
                nc.sync.dma_start(out=scratch[sp, 2 * p:2 * p + 2, sl],
                                  in_=denp[2 * half:2 * half + 2, :])
                for h in range(2):
                    rb = rbp.tile([64, SPAN], F32, tag=f"rb{h}", name=f"rb{h}")
                    nc.sync.dma_start(
                        out=rb[:],
                        in_=scratch[sp, 2 * p + h, sl].partition_broadcast(64)
                    )
                    if h == 0:
                        nc.vector.tensor_mul(
                            avc[(p, sp)][0:64, sl], avs[(p, 0, sp)][:, sl], rb[:])
                    else:
                        nc.vector.tensor_mul(
                            avs[(p, 1, sp)][:, sl], avs[(p, 1, sp)][:, sl], rb[:])
                        nc.sync.dma_start(
                            out=avc[(p, sp)][64:128, sl],
                            in_=avs[(p, 1, sp)][:, sl],
                        )

        def oproj_sp(sp):
            # o-proj with the Wo slice stationary, producing the TRANSPOSED
            # output out_T[d, q] = sum_f wo[f, d] * avc[f, q] (host untransposes).
            # Each d-chunk reuses one weight load across both q-spans per
            # pair; drain copies alternate DVE/ACT so neither engine is the
            # tail bottleneck.
            for dch in range(NDC):
                pool, tag = [(accp, "av0"), (accp, "av1"),
                             (mmp, "mm"), (mmp, "mm")][dch % 4]
                o_ps = pool.tile([128, 2 * SPAN], F32, tag=tag, name="ops")
                for p in range(2):
                    for j in range(2):
                        nc.tensor.matmul(
                            o_ps[:, j * SPAN:(j + 1) * SPAN],
                            wo2_sb[:, p, dch * 128:(dch + 1) * 128],
                            avc[(p, sp)][:, j * SPAN:(j + 1) * SPAN],
                            start=(p == 0), stop=(p == 1),
                        )
                o_sbT = outp.tile([128, 2 * SPAN], BF16, tag="osb", name="osb")
                if dch % 2 == 0:
                    nc.vector.tensor_copy(o_sbT[:], o_ps[:])
                else:
                    nc.scalar.copy(o_sbT[:], o_ps[:])
                nc.sync.dma_start(
                    out=out[dch * 128:(dch + 1) * 128,
                            sp * 2 * SPAN:(sp + 1) * 2 * SPAN],
                    in_=o_sbT[:])

        # ---- schedule ----
        # minimal prefix so the first exp lands early: k0, q0, q1 feed the
        # first group's scores; remaining k/v/q spans ride inside the first
        # two groups' chunk loops.
        xk0_t = xin.tile([128, NDC, SPAN], BF16, tag="xk", name="xk", bufs=2)
        nc.sync.dma_start(out=xk0_t[:, 0:2, :], in_=xk[:, 0, 0:2, :])
        nc.sync.dma_start(out=wk_sb[:, 2:NDC, :], in_=wk[:, 2:NDC, :])
        nc.sync.dma_start(out=xk0_t[:, 2:NDC, :], in_=xk[:, 0, 2:NDC, :])
        nc.sync.dma_start(out=wq_sb[:], in_=wq[:])
        nc.sync.dma_start(out=bk_sb[:], in_=bkt[:])
        nc.sync.dma_start(out=bq_sb[:], in_=bqt[:])
        kproj(0, xk0_t)
        qproj(0)
        qproj(1)
        nc.sync.dma_start(out=wv_sb[:], in_=wv[:])
        nc.sync.dma_start(out=bv_sb[:], in_=bvt[:])
        vproj(0)
        # group (0,0): one small projection unit per chunk so the single-exp
        # lookahead amortizes the PSUM mm-slot each unit borrows.  Weight
        # producers stay >= 1 chunk ahead of their consuming matmuls.
        xk_t = {}
        xv_t = {}
        units = {
            1: [("k", 1, 0)], 2: [("k", 1, 1)],
            3: [("v", 1, 0)], 4: [("v", 1, 1)], 5: [("v", 1, 2)],
            6: [("v", 1, 3), ("k", 2, 0)],
            7: [("k", 2, 1), ("v", 2, 0)],
            8: [("v", 2, 1)], 9: [("v", 2, 2)],
            10: [("v", 2, 3), ("k", 3, 0)],
            11: [("k", 3, 1), ("v", 3, 0)],
            12: [("v", 3, 1)], 13: [("v", 3, 2)],
            14: [("v", 3, 3), ("q", 2, 0)],
            15: [("q", 2, 1), ("q", 3, 0)],
        }
        for c in range(NKC):
            for kind, s, i in units.get(c, []):
                if kind == "k":
                    if i == 0:
                        xk_t[s] = dma_x(xk, "xk", s)
                    fproj_pair(wk_sb, bk_sb, xk_t[s], khT, s, i)
                elif kind == "q":
                    if i == 0:
                        xk_t[("q", s)] = dma_x(xq, "xq", s)
                    fproj_pair(wq_sb, bq_sb, xk_t[("q", s)], qhT, s, i)
                else:
                    if i == 0:
                        xv_t[s] = dma_x(xv, "xv", s)
                    vproj_tt(xv_t[s], s, i)
            attn_chunks(0, 0, c, c + 1)
        normalize(0, 0)
        # finish q3 (second pair) before the transition work
        fproj_pair(wq_sb, bq_sb, xk_t[("q", 3)], qhT, 3, 1)
        attn_chunks(0, 1, 0, 16)
        normalize(0, 1)
        nc.sync.dma_start(out=wo2_sb[:], in_=wo[:])
        attn_chunks(1, 0, 0, 16)
        normalize(1, 0)
        attn_chunks(1, 1, 0, 16)
        # tail: the sp0 o-proj units flow immediately (avc ready) while the
        # normalize(1, 1) chain runs; sp1 units follow as avc(., 1) lands
        normalize(1, 1)
        oproj_sp(0)
        oproj_sp(1)


def shard_inputs(q, k, v, Wq, bq, Wk, bk, Wv, bv, Wo, bo):
    """Host-side shard + layout prep. Returns list of 8 per-core input dicts."""
    def chunk_pf(a2d, pdim, dtype=np.float32):
        # (n*pdim, f) -> (pdim, n, f) with row r = o*pdim + p
        n, f = a2d.shape
        return np.ascontiguousarray(
            a2d.reshape(n // pdim, pdim, f).transpose(1, 0, 2)
        ).astype(dtype)

    def chunk_span(a2d, dtype):
        # (1024, 2048) -> [128, NSPAN, NDC, SPAN] so each span slice is a
        # contiguous 8 KiB per partition (fast DMA)
        x = chunk_pf(a2d, 128, dtype)          # [128, NDC, L]
        return np.ascontiguousarray(
            x.reshape(128, NDC, NSPAN, SPAN).transpose(0, 2, 1, 3))

    in_maps = []
    for core in range(N_CORES):
        b = core // GROUPS
        g = core % GROUPS
        fs = slice(g * F, (g + 1) * F)
        bf = ml_dtypes.bfloat16
        m = {
            "xq": chunk_span(np.ascontiguousarray(q[b].T), bf),
            "xk": chunk_span(np.ascontiguousarray(k[b].T), bf),
            "xv": chunk_span(np.ascontiguousarray(v[b].T), bf),
            "wq": chunk_pf(np.ascontiguousarray(Wq[fs, :].T), 128, bf),
            "wk": chunk_pf(np.ascontiguousarray(Wk[fs, :].T), 128, bf),
            "wv": chunk_pf(np.ascontiguousarray(Wv[fs, :].T), 128, bf),
            "wo": chunk_pf(np.ascontiguousarray(Wo[:, fs].T), 128, bf),
            "bqt": np.ascontiguousarray(bq[fs].reshape(2, 128).T).astype(np.float32),
            "bkt": np.ascontiguousarray(bk[fs].reshape(2, 128).T).astype(np.float32),
            "bvt": np.ascontiguousarray(bv[fs].reshape(1, F)).astype(ml_dtypes.bfloat16),
        }
        in_maps.append(m)
    return in_maps


_NC_CACHE = None


def _get_nc():
    global _NC_CACHE
    if _NC_CACHE is None:
        _NC_CACHE = build_bass()
    return _NC_CACHE


def run_spmd(inputs, trace=False, **kw):
    """Run the 8-core kernel; returns (full_output, BassKernelResults)."""
    q = np.asarray(inputs["q"], np.float32)
    k = np.asarray(inputs["k"], np.float32)
    v = np.asarray(inputs["v"], np.float32)
    in_maps = shard_inputs(
        q, k, v,
        np.asarray(inputs["Wq"], np.float32), np.asarray(inputs["bq"], np.float32),
        np.asarray(inputs["Wk"], np.float32), np.asarray(inputs["bk"], np.float32),
        np.asarray(inputs["Wv"], np.float32), np.asarray(inputs["bv"], np.float32),
        np.asarray(inputs["Wo"], np.float32), np.asarray(inputs["bo"], np.float32),
    )
    nc = _get_nc()
    res = run_bass_kernel_spmd(nc, in_maps, core_ids=list(range(N_CORES)),
                               trace=trace, **kw)
    bo = np.asarray(inputs["bo"], np.float32)
    full = np.empty((B, L, D), np.float32)
    for b in range(B):
        acc = res.results[b * GROUPS]["out"].astype(np.float32)
        for g in range(1, GROUPS):
            acc = acc + res.results[b * GROUPS + g]["out"].astype(np.float32)
        full[b] = acc.T + bo[None, :]   # device output is [D, L]
    return full, res


def kernel(**inputs):
    kpm = np.asarray(inputs["key_padding_mask"])
    if not bool(kpm.all()):
        return _numpy_fallback(**inputs)
    out, _ = run_spmd(inputs)
    return out


def _numpy_fallback(q, k, v, key_padding_mask, Wq, bq, Wk, bk, Wv, bv, Wo, bo):
    q = np.asarray(q, np.float32)
    k = np.asarray(k, np.float32)
    v = np.asarray(v, np.float32)
    B_, Lq, D_ = q.shape
    qh = (q @ np.asarray(Wq).T + bq).reshape(B_, Lq, H, DH).transpose(0, 2, 1, 3)
    kh = (k @ np.asarray(Wk).T + bk).reshape(B_, -1, H, DH).transpose(0, 2, 1, 3)
    vh = (v @ np.asarray(Wv).T + bv).reshape(B_, -1, H, DH).transpose(0, 2, 1, 3)
    s = np.einsum("bhqd,bhkd->bhqk", qh, kh) / np.sqrt(np.float32(DH))
    km = np.asarray(key_padding_mask)[:, None, None, :]
    s = np.where(km, s, -np.inf)
    s = s - s.max(-1, keepdims=True)
    p = np.exp(s)
    p = p / p.sum(-1, keepdims=True)
    o = np.einsum("bhqk,bhkd->bhqd", p, vh)
    o = o.transpose(0, 2, 1, 3).reshape(B_, Lq, D_)
    return (o @ np.asarray(Wo).T + bo).astype(np.float32)
